# revision 1
# baseline (speedup 1.0000x reference)
"""DenseEquivariantFFT Trainium2 kernel (batch-sharded over 8 cores).

Math: y = IDFT2_g( sum_{i,s1} DFT2_g(x)[b,i,s1,f] * DFT2_g(kernel[..,mapping])[o,i,s1,s2,f] ) + bias
Device computes, per core (128 batches): real-basis 2D-DFT over cells (as
matmuls), per-frequency (i,s1)->(o,s2) mixing matmuls, inverse DFT.
Weight-side DFT + gather are host-precomputed from the small kernel tensor.
"""
import numpy as np
import ml_dtypes

N_CORES = 8
B, CIN, COUT, NS, NCELL, G = 1024, 32, 32, 8, 64, 512
BC = B // N_CORES  # 128 batches per core

_CACHE = {}


def _freq_classes():
    singles, reps = [], []
    for ku in range(8):
        for kv in range(8):
            f = ku * 8 + kv
            cf = ((-ku) % 8) * 8 + ((-kv) % 8)
            if cf == f:
                singles.append(f)
            elif f < cf:
                reps.append(f)
    return singles, reps  # 4, 30


def _transforms():
    singles, reps = _freq_classes()
    u, v = np.meshgrid(np.arange(8), np.arange(8), indexing="ij")

    def theta(f):
        ku, kv = divmod(f, 8)
        return 2 * np.pi * (ku * u + kv * v) / 8

    Cf = np.zeros((64, 64))
    Ci = np.zeros((64, 64))
    for j, f in enumerate(singles):
        Cf[:, j] = np.cos(theta(f)).ravel()
        Ci[j, :] = np.cos(theta(f)).ravel() / 64
    for j, f in enumerate(reps):
        Cf[:, 4 + 2 * j] = np.cos(theta(f)).ravel()
        Cf[:, 5 + 2 * j] = -np.sin(theta(f)).ravel()
        Ci[4 + 2 * j, :] = 2 * np.cos(theta(f)).ravel() / 64
        Ci[5 + 2 * j, :] = -2 * np.sin(theta(f)).ravel() / 64
    return Cf, Ci, singles, reps


def host_constants(kern, bias, mapping):
    """Build device weight tensors from kernel/bias/mapping (any mapping)."""
    Cf, Ci, singles, reps = _transforms()
    Kexp = kern[:, :, mapping.reshape(NS, NS, NCELL)]  # [o,i,s1,s2,c]
    KF = np.fft.fft2(
        Kexp.reshape(COUT, CIN, NS, NS, 8, 8).astype(np.float64), axes=(-2, -1)
    ).reshape(COUT, CIN, NS, NS, NCELL)

    def rq(a):  # [o,i,s1,s2] -> [r=(s1,i), q=(o,s2)]
        return a.transpose(2, 1, 0, 3).reshape(NS * CIN, COUT * NS)

    W_pairs = np.zeros((120, 128, 512), np.float32)
    for j, f in enumerate(reps):
        kr, ki = rq(KF[..., f].real), rq(KF[..., f].imag)
        for h in range(2):
            rs = slice(128 * h, 128 * h + 128)
            W_pairs[(h * 2 + 0) * 30 + j] = np.concatenate(
                [kr[rs], ki[rs]], axis=1
            )
            W_pairs[(h * 2 + 1) * 30 + j] = np.concatenate(
                [-ki[rs], kr[rs]], axis=1
            )
    W_singles = np.zeros((8, 128, 256), np.float32)
    for j, f in enumerate(singles):
        kr = rq(KF[..., f].real)
        for h in range(2):
            W_singles[h * 4 + j] = kr[128 * h : 128 * h + 128]
    bias_row = 64.0 * np.repeat(bias.ravel().astype(np.float64), 8)[None, :]
    bf = ml_dtypes.bfloat16
    return {
        "Cf": Cf.astype(bf),
        "Ci2": np.kron(np.eye(2), Ci).astype(bf),  # [(q2,fc),(q2,gam)] 128x128
        "W_pairs": W_pairs.astype(bf),
        "W_singles": W_singles.astype(bf),
        "bias_row": bias_row.astype(bf),
        "ident": np.eye(128).astype(bf),
        "ones1": np.ones((1, 128), bf),
    }


def host_simulate(x, kern, bias, mapping):
    """Pure-numpy simulation of the exact device math (for validation)."""
    Cf, Ci, singles, reps = _transforms()
    c = host_constants(kern, bias, mapping)
    xs = x.reshape(B, CIN, NCELL, NS).transpose(0, 3, 1, 2)  # [b,s1,i,c]
    XF = xs.astype(np.float64) @ Cf  # [b,s1,i,fc]
    XF2 = XF.transpose(0, 1, 2, 3).reshape(B, NS * CIN, 64)  # r=(s1,i)
    YF = np.zeros((B, COUT * NS, 64))
    Wp = c["W_pairs"].astype(np.float64)
    Ws = c["W_singles"].astype(np.float64)
    for j in range(30):
        acc = np.zeros((B, 512))
        for h in range(2):
            rs = slice(128 * h, 128 * h + 128)
            acc += XF2[:, rs, 4 + 2 * j] @ Wp[(h * 2 + 0) * 30 + j]
            acc += XF2[:, rs, 5 + 2 * j] @ Wp[(h * 2 + 1) * 30 + j]
        YF[:, :, 4 + 2 * j] = acc[:, :256]
        YF[:, :, 5 + 2 * j] = acc[:, 256:]
    for j in range(4):
        acc = np.zeros((B, 256))
        for h in range(2):
            rs = slice(128 * h, 128 * h + 128)
            acc += XF2[:, rs, j] @ Ws[h * 4 + j]
        if j == 0:
            acc = acc + c["bias_row"].astype(np.float64)
        YF[:, :, j] = acc
    y = np.einsum("bqf,fg->bqg", YF, Ci)  # [b,(o,s2),gam]
    # q=(o,s2), cols -> y[b,o,gam*8+s2]
    y = y.reshape(B, COUT, NS, NCELL).transpose(0, 1, 3, 2).reshape(B, COUT, G)
    return y.astype(np.float32)


def _copy(eng, out, in_):
    if hasattr(eng, "tensor_copy"):
        eng.tensor_copy(out, in_)
    else:
        eng.copy(out, in_)


def _build_program():
    import concourse.bass as bass
    import concourse.bacc as bacc
    import concourse.mybir as mybir
    from concourse.tile import TileContext

    BF = mybir.dt.bfloat16
    F32 = mybir.dt.float32
    nc = bacc.Bacc("TRN2", target_bir_lowering=False, debug=False,
                   num_devices=N_CORES)
    x_d = nc.dram_tensor("x", [BC, CIN * G], F32, kind="ExternalInput")
    cf_d = nc.dram_tensor("Cf", [64, 64], BF, kind="ExternalInput")
    ci_d = nc.dram_tensor("Ci2", [128, 128], BF, kind="ExternalInput")
    wp_d = nc.dram_tensor("W_pairs", [120, 128, 512], BF, kind="ExternalInput")
    ws_d = nc.dram_tensor("W_singles", [8, 128, 256], BF, kind="ExternalInput")
    br_d = nc.dram_tensor("bias_row", [1, 256], BF, kind="ExternalInput")
    id_d = nc.dram_tensor("ident", [128, 128], BF, kind="ExternalInput")
    on_d = nc.dram_tensor("ones1", [1, 128], BF, kind="ExternalInput")
    y_d = nc.dram_tensor("y", [BC, CIN * G], F32, kind="ExternalOutput")

    xr = x_d.ap().rearrange("b (i g) -> (b i) g", g=G).rearrange(
        "(t p) g -> t p g", p=128
    )  # 32 tiles [128=(b4,i32), 512=(c,s)]

    with TileContext(nc) as tc:
        with (
            tc.tile_pool(name="const", bufs=1) as cpool,
            tc.tile_pool(name="xt", bufs=1) as xtpool,
            tc.tile_pool(name="xf2", bufs=1) as xfpool,
            tc.tile_pool(name="yf", bufs=1) as yfpool,
            tc.tile_pool(name="x0", bufs=4) as x0pool,
            tc.tile_pool(name="w", bufs=6) as wpool,
            tc.tile_pool(name="ev", bufs=6) as evpool,
            tc.tile_pool(name="yout", bufs=2) as yopool,
            tc.tile_pool(name="ps_s", bufs=2, space="PSUM") as ps_s,
            tc.tile_pool(name="ps_c", bufs=1, space="PSUM") as ps_c,
            tc.tile_pool(name="ps_m", bufs=2, space="PSUM") as ps_m,
            tc.tile_pool(name="ps_e", bufs=1, space="PSUM") as ps_e,
        ):
            cf_s = cpool.tile([64, 64], BF)
            nc.sync.dma_start(out=cf_s[:, :], in_=cf_d[:, :])
            ci_s = cpool.tile([128, 128], BF)
            nc.sync.dma_start(out=ci_s[:, :], in_=ci_d[:, :])
            br_s = cpool.tile([1, 256], BF)
            nc.sync.dma_start(out=br_s[:, :], in_=br_d[:, :])
            id_s = cpool.tile([128, 128], BF)
            nc.sync.dma_start(out=id_s[:, :], in_=id_d[:, :])
            on_s = cpool.tile([1, 128], BF)
            nc.sync.dma_start(out=on_s[:, :], in_=on_d[:, :])

            # XT [64 c, (t,s,bi)=32768] bf16 ; XF2 [128 r, (b,fc)=8192] x2
            xt = xtpool.tile([64, 32768], BF)
            xf2 = [xfpool.tile([128, 8192], BF, name=f"xf2_{h}", tag=f"xf{h}")
                   for h in range(2)]
            yf = yfpool.tile([128, 16384], BF)

            # Stage A+B: load tiles, transpose c onto partitions
            for t in range(32):
                x0 = x0pool.tile([128, 512], BF, tag="x0")
                nc.gpsimd.dma_start(out=x0[:, :], in_=xr[t])  # f32->bf16 cast
                x0r = x0[:, :].rearrange("p (c s) -> p s c", s=8)
                for s0 in range(8):
                    pt = ps_s.tile([64, 128], BF, tag="pB")
                    nc.tensor.transpose(pt[:, :], x0r[:, s0], id_s[:, :])
                    eng = nc.scalar if (s0 % 2) else nc.vector
                    dst = xt[:, :].rearrange(
                        "c (t b4 s i) -> c t s b4 i", t=32, b4=4, s=8
                    )
                    _copy(eng, dst[:, t, s0], pt[:, :].rearrange(
                        "c (b4 i) -> c b4 i", b4=4))

            # Stage C: forward DFT -> XF2[r=(s_idx,i), b*64+fc]
            xtr = xt[:, :].rearrange(
                "c (tb h r) -> c tb h r", h=2, r=128
            )
            for babs in range(128):
                for h in range(2):
                    pf = ps_c.tile([128, 64], F32, tag="pC")
                    lhsT = xtr[:, babs, h, :]
                    nc.tensor.matmul(
                        pf[:, :], lhsT, cf_s[:, :], start=True, stop=True
                    )
                    eng = nc.scalar if (babs % 2) else nc.vector
                    dst = xf2[h][:, :].rearrange("r (b f) -> r b f", f=64)
                    _copy(eng, dst[:, babs, :], pf[:, :])

            # Stage D: per-frequency mixing
            xf2r = [xf2[h][:, :].rearrange("r (b f) -> r b f", f=64)
                    for h in range(2)]
            yfr = yf[:, :].rearrange("b (q f) -> b q f", f=64)
            for j in range(30):
                pm = ps_m.tile([128, 512], F32, tag="pD")
                k = 0
                for h in range(2):
                    for ci in range(2):
                        w = wpool.tile([128, 512], BF, tag="wp")
                        nc.sync.dma_start(
                            out=w[:, :], in_=wp_d[(h * 2 + ci) * 30 + j]
                        )
                        nc.tensor.matmul(
                            pm[:, :],
                            xf2r[h][:, :, 4 + 2 * j + ci],
                            w[:, :],
                            start=(k == 0),
                            stop=(k == 3),
                        )
                        k += 1
                eng = nc.scalar if (j % 2) else nc.vector
                _copy(eng, yfr[:, :, 4 + 2 * j], pm[:, 0:256])
                eng2 = nc.vector if (j % 2) else nc.scalar
                _copy(eng2, yfr[:, :, 5 + 2 * j], pm[:, 256:512])
            for j in range(4):
                pm = ps_m.tile([128, 256], F32, tag="pD")
                nmm = 3 if j == 0 else 2
                k = 0
                for h in range(2):
                    w = wpool.tile([128, 256], BF, tag="wsg")
                    nc.sync.dma_start(out=w[:, :], in_=ws_d[h * 4 + j])
                    nc.tensor.matmul(
                        pm[:, :],
                        xf2r[h][:, :, j],
                        w[:, :],
                        start=(k == 0),
                        stop=(k == nmm - 1),
                    )
                    k += 1
                if j == 0:
                    nc.tensor.matmul(
                        pm[:, :], on_s[:, :], br_s[:, :],
                        start=False, stop=True,
                    )
                eng = nc.scalar if (j % 2) else nc.vector
                _copy(eng, yfr[:, :, j], pm[:, :])

            # Stage E: inverse DFT + transpose back + store
            for w_ in range(8):
                yo = yopool.tile([128, 2048], F32, tag="yo")
                yov = yo[:, :].rearrange("b (o g s) -> b o g s", o=4, s=8)
                for pp in range(16):
                    P = 16 * w_ + pp
                    o, s2_0 = divmod(2 * P, 8)
                    ptile = ps_e.tile([128, 128], BF, tag="pE1")
                    src = yf[:, :].rearrange(
                        "b (qp q2 f) -> b qp q2 f", q2=2, f=64
                    )
                    nc.tensor.transpose(ptile[:, :], src[:, P], id_s[:, :])
                    yt = evpool.tile([128, 128], BF, tag="yt")
                    eng = nc.scalar if (pp % 2) else nc.vector
                    _copy(eng, yt[:, :], ptile[:, :])
                    pi = ps_e.tile([128, 128], F32, tag="pE2")
                    nc.tensor.matmul(
                        pi[:, :], ci_s[:, :], yt[:, :], start=True, stop=True
                    )
                    yi = evpool.tile([128, 128], BF, tag="yi")
                    eng2 = nc.vector if (pp % 2) else nc.scalar
                    _copy(eng2, yi[:, :], pi[:, :])
                    pt2 = ps_e.tile([128, 128], BF, tag="pE3")
                    nc.tensor.transpose(pt2[:, :], yi[:, :], id_s[:, :])
                    # cols of pt2 = (q2, gam); write y[b, o%8, gam, s2_0+q2]
                    pv = pt2[:, :].rearrange("b (q2 g) -> b q2 g", q2=2)
                    eng3 = nc.scalar if (pp % 2) else nc.vector
                    for q2 in range(2):
                        _copy(eng3,
                            yov[:, o % 4, :, s2_0 + q2], pv[:, q2, :]
                        )
                nc.sync.dma_start(
                    out=y_d.ap()[:, w_ * 2048 : (w_ + 1) * 2048], in_=yo[:, :]
                )
    nc.compile()
    return nc


def kernel(**inputs):
    x = np.asarray(inputs["x"], np.float32)
    kern = np.asarray(inputs["kernel"], np.float32)
    bias = np.asarray(inputs["bias"], np.float32)
    mapping = np.asarray(inputs["mapping"])
    from concourse.bass_utils import run_bass_kernel_spmd

    if "nc" not in _CACHE:
        _CACHE["nc"] = _build_program()
    nc = _CACHE["nc"]
    consts = host_constants(kern, bias, mapping)
    in_maps = []
    for c in range(N_CORES):
        m = dict(consts)
        m["x"] = np.ascontiguousarray(
            x[c * BC : (c + 1) * BC].reshape(BC, CIN * G)
        )
        in_maps.append(m)
    res = run_bass_kernel_spmd(nc, in_maps, list(range(N_CORES)))
    _CACHE["last_exec_ns"] = res.exec_time_ns
    y = np.concatenate(
        [res.results[c]["y"].reshape(BC, CIN, G) for c in range(N_CORES)], 0
    )
    return y.astype(np.float32)



# revision 4
# speedup vs baseline: 1.2882x; 1.2882x over previous
"""DenseEquivariantFFT Trainium2 kernel (batch-sharded over 8 cores), v2.

Math: y = IDFT2_cells( sum_{i,s1} DFT2_cells(x)[b,i,s1,f] * KF[o,i,s1,s2,f] ) + bias
where KF = DFT2_cells(kernel[..,mapping]) and f runs over the 64 cell
frequencies in a real (cos/sin) basis.

Device dataflow per core (128 batches), all bf16 with f32 PSUM accum:
 - host pre-transposes x into [(s1-parity, cell), (t, b4, sp, i)] layout,
   so no on-device input transposes are needed.
 - stage C: per batch, one matmul [K=128=(par,c)] x blockdiag(Cf) -> XF
   with partitions (sp,i) and free (batch, parity, fc).
 - stage D: per frequency pair, 4 matmuls [K=128, N=512] against
   deduplicated [kr|ki] weights; re/im recombined on the vector engine.
 - stage E: DMA-xbar transposes (128x128) put fc on partitions, then one
   matmul per chunk against blockdiag(Ci) produces spatial output with
   batch back on partitions; host un-permutes the stored layout.
"""
import numpy as np
import ml_dtypes

N_CORES = 8
B, CIN, COUT, NS, NCELL, G = 1024, 32, 32, 8, 64, 512
BC = B // N_CORES  # 128 batches per core

_CACHE = {}


def _freq_classes():
    singles, reps = [], []
    for ku in range(8):
        for kv in range(8):
            f = ku * 8 + kv
            cf = ((-ku) % 8) * 8 + ((-kv) % 8)
            if cf == f:
                singles.append(f)
            elif f < cf:
                reps.append(f)
    return singles, reps  # 4, 30


def _transforms():
    singles, reps = _freq_classes()
    u, v = np.meshgrid(np.arange(8), np.arange(8), indexing="ij")

    def theta(f):
        ku, kv = divmod(f, 8)
        return 2 * np.pi * (ku * u + kv * v) / 8

    Cf = np.zeros((64, 64))
    Ci = np.zeros((64, 64))
    for j, f in enumerate(singles):
        Cf[:, j] = np.cos(theta(f)).ravel()
        Ci[j, :] = np.cos(theta(f)).ravel() / 64
    for j, f in enumerate(reps):
        Cf[:, 4 + 2 * j] = np.cos(theta(f)).ravel()
        Cf[:, 5 + 2 * j] = -np.sin(theta(f)).ravel()
        Ci[4 + 2 * j, :] = 2 * np.cos(theta(f)).ravel() / 64
        Ci[5 + 2 * j, :] = -2 * np.sin(theta(f)).ravel() / 64
    return Cf, Ci, singles, reps


def host_constants(kern, bias, mapping):
    """Device weight tensors. W rows use r=(sp,i) with s1=2*sp+h (parity
    halves); W cols use q=(s2,o)."""
    Cf, Ci, singles, reps = _transforms()
    Kexp = kern[:, :, mapping.reshape(NS, NS, NCELL)]  # [o,i,s1,s2,c]
    KF = np.fft.fft2(
        Kexp.reshape(COUT, CIN, NS, NS, 8, 8).astype(np.float64), axes=(-2, -1)
    ).reshape(COUT, CIN, NS, NS, NCELL)

    wp = np.zeros((64, 128, 512), np.float64)  # unit = 2*j + h
    for j, f in enumerate(reps):
        A = KF[..., f]  # [o,i,s1,s2]
        krf = A.real.transpose(2, 1, 3, 0).reshape(NS, CIN, NS * COUT)
        kif = A.imag.transpose(2, 1, 3, 0).reshape(NS, CIN, NS * COUT)
        for h in range(2):
            kr = krf[h::2].reshape(128, 256)
            ki = kif[h::2].reshape(128, 256)
            wp[2 * j + h] = np.concatenate([kr, ki], axis=1)
    ws = np.zeros((8, 128, 256), np.float64)  # unit = 2*js + h
    for js, f in enumerate(singles):
        A = KF[..., f].real.transpose(2, 1, 3, 0).reshape(NS, CIN, NS * COUT)
        for h in range(2):
            ws[2 * js + h] = A[h::2].reshape(128, 256)

    bias_row = 64.0 * np.tile(bias.ravel().astype(np.float64), NS)[None, :]
    bf = ml_dtypes.bfloat16
    return {
        "CfK": np.kron(np.eye(2), Cf).astype(bf),          # [128,128]
        "CiK": np.kron(np.eye(2), Ci).astype(bf),          # [128,128]
        "Wp": np.ascontiguousarray(
            wp.reshape(4, 16, 128, 512).transpose(0, 2, 1, 3)
        ).reshape(4, 128, 16 * 512).astype(bf),
        "Ws": np.ascontiguousarray(
            ws.transpose(1, 0, 2)
        ).reshape(128, 8 * 256).astype(bf),
        "bias_row": bias_row.astype(bf),
        "ones1": np.ones((1, 128), bf),
    }


def host_prep_x(xc):
    """[128,32,512] f32 -> [128=(par,c), 16384=(t,b4,sp,i)] bf16."""
    xs = xc.reshape(32, 4, CIN, NCELL, 4, 2)  # t,b4,i,c,sp,par
    xt2 = xs.transpose(5, 3, 0, 1, 4, 2).reshape(128, 16384)
    return np.ascontiguousarray(xt2.astype(ml_dtypes.bfloat16))


def host_unpack_y(yo):
    """[128, 16384=(qp,q2,c)] bf16 -> [128, 32, 512] f32; q=(s2,o)=2qp+q2."""
    arr = np.asarray(yo, np.float32).reshape(BC, 256, 64)      # b, q, c
    arr = arr.reshape(BC, NS, COUT, NCELL).transpose(0, 2, 3, 1)  # b,o,c,s2
    return np.ascontiguousarray(arr).reshape(BC, COUT, G)


def host_simulate(x, kern, bias, mapping):
    """f64 numpy mirror of the device algebra (layout validation)."""
    Cf, Ci, singles, reps = _transforms()
    Kexp = kern[:, :, mapping.reshape(NS, NS, NCELL)]
    KF = np.fft.fft2(
        Kexp.reshape(COUT, CIN, NS, NS, 8, 8).astype(np.float64), axes=(-2, -1)
    ).reshape(COUT, CIN, NS, NS, NCELL)
    xs = x.reshape(B, CIN, NCELL, NS).astype(np.float64)
    XF = np.einsum("bics,cf->bisf", xs, Cf)  # [b,i,s1,fc]
    yf = np.zeros((B, NS, COUT, 64))  # [b,s2,o,fc]
    for j, f in enumerate(reps):
        A = KF[..., f]
        yf[..., 4 + 2 * j] = (
            np.einsum("bis,oist->bto", XF[..., 4 + 2 * j], A.real)
            - np.einsum("bis,oist->bto", XF[..., 5 + 2 * j], A.imag)
        )
        yf[..., 5 + 2 * j] = (
            np.einsum("bis,oist->bto", XF[..., 4 + 2 * j], A.imag)
            + np.einsum("bis,oist->bto", XF[..., 5 + 2 * j], A.real)
        )
    for js, f in enumerate(singles):
        yf[..., js] = np.einsum("bis,oist->bto", XF[..., js], KF[..., f].real)
    yf[..., 0] += 64.0 * bias.ravel()[None, None, :]
    y = np.einsum("btof,fc->btoc", yf, Ci)  # [b,s2,o,c]
    y = y.transpose(0, 2, 3, 1).reshape(B, COUT, G)
    return y.astype(np.float32)


def _build_program():
    import concourse.bass as bass
    import concourse.bacc as bacc
    import concourse.mybir as mybir
    from concourse.tile import TileContext

    BF = mybir.dt.bfloat16
    F32 = mybir.dt.float32
    nc = bacc.Bacc("TRN2", target_bir_lowering=False, debug=False,
                   num_devices=N_CORES)
    x_d = nc.dram_tensor("x", [128, 16384], BF, kind="ExternalInput")
    cfk_d = nc.dram_tensor("CfK", [128, 128], BF, kind="ExternalInput")
    cik_d = nc.dram_tensor("CiK", [128, 128], BF, kind="ExternalInput")
    wp_d = nc.dram_tensor("Wp", [4, 128, 8192], BF, kind="ExternalInput")
    ws_d = nc.dram_tensor("Ws", [128, 2048], BF, kind="ExternalInput")
    br_d = nc.dram_tensor("bias_row", [1, 256], BF, kind="ExternalInput")
    on_d = nc.dram_tensor("ones1", [1, 128], BF, kind="ExternalInput")
    y_d = nc.dram_tensor("y", [128, 16384], BF, kind="ExternalOutput")

    with TileContext(nc) as tc:
        with (
            tc.tile_pool(name="const", bufs=1) as cpool,
            tc.tile_pool(name="xt", bufs=1) as xtpool,
            tc.tile_pool(name="xf3", bufs=1) as xfpool,
            tc.tile_pool(name="w", bufs=1) as wpool,
            tc.tile_pool(name="yf", bufs=1) as yfpool,
            tc.tile_pool(name="yo", bufs=1) as yopool,
            tc.tile_pool(name="pbc", bufs=3) as pbcpool,
            tc.tile_pool(name="yft", bufs=6) as ytpool,
            tc.tile_pool(name="ps", bufs=2, space="PSUM") as pspool,
        ):
            cfk = cpool.tile([128, 128], BF, name="cfk")
            nc.sync.dma_start(out=cfk[:, :], in_=cfk_d[:, :])
            cik = cpool.tile([128, 128], BF, name="cik")
            nc.sync.dma_start(out=cik[:, :], in_=cik_d[:, :])
            br = cpool.tile([1, 256], BF, name="br")
            nc.sync.dma_start(out=br[:, :], in_=br_d[:, :])
            on = cpool.tile([1, 128], BF, name="on")
            nc.sync.dma_start(out=on[:, :], in_=on_d[:, :])

            xt = [xtpool.tile([128, 4096], BF, name=f"xt{g}", tag=f"xt{g}")
                  for g in range(4)]
            for g in range(4):
                nc.sync.dma_start(
                    out=xt[g][:, :], in_=x_d[:, 4096 * g: 4096 * (g + 1)]
                )
            # weights on the ACT HWDGE ring, parallel with x loads
            wsb = [wpool.tile([128, 8192], BF, name=f"wp{k}", tag=f"wp{k}")
                   for k in range(4)]
            for k in range(4):
                nc.scalar.dma_start(out=wsb[k][:, :], in_=wp_d[k])
            ws = wpool.tile([128, 2048], BF, name="ws", tag="ws")
            nc.scalar.dma_start(out=ws[:, :], in_=ws_d[:, :])

            xf3 = xfpool.tile([128, 16384], BF, name="xf3")
            yf = yfpool.tile([128, 16384], BF, name="yf")
            yo = yopool.tile([128, 16384], BF, name="yo")

            # ---- stage C: forward cell-DFT, one matmul per batch ----
            for t in range(32):
                g, tl = divmod(t, 8)
                xtr = xt[g][:, :].rearrange(
                    "p (t b4 r) -> p t b4 r", t=8, b4=4
                )
                pc = pspool.tile([128, 512], F32, name="pc", tag="psC")
                for b4 in range(4):
                    nc.tensor.matmul(
                        pc[:, 128 * b4: 128 * b4 + 128],
                        xtr[:, tl, b4, :],
                        cfk[:, :],
                        start=True, stop=True,
                    )
                dst = xf3[:, :].rearrange("p (t q) -> p t q", t=32)
                if t % 2:
                    nc.scalar.copy(dst[:, t, :], pc[:, :])
                else:
                    nc.vector.tensor_copy(dst[:, t, :], pc[:, :])

            # ---- stage D: per-frequency mixing ----
            xf3r = xf3[:, :].rearrange(
                "p (b s2 fc) -> p s2 fc b", s2=2, fc=64
            )
            yfr = yf[:, :].rearrange("p (q fc) -> p q fc", fc=64)
            for j in range(30):
                k, unit0 = divmod(2 * j, 16)
                pa = pspool.tile([128, 512], F32, name="pa", tag="pa")
                pb = pspool.tile([128, 512], F32, name="pb", tag="pb")
                for h in range(2):
                    rhs = wsb[k][:, 512 * (unit0 + h): 512 * (unit0 + h) + 512]
                    nc.tensor.matmul(
                        pa[:, :], xf3r[:, h, 4 + 2 * j, :], rhs,
                        start=(h == 0), stop=(h == 1),
                    )
                for h in range(2):
                    rhs = wsb[k][:, 512 * (unit0 + h): 512 * (unit0 + h) + 512]
                    nc.tensor.matmul(
                        pb[:, :], xf3r[:, h, 5 + 2 * j, :], rhs,
                        start=(h == 0), stop=(h == 1),
                    )
                pbc = pbcpool.tile([128, 512], BF, name="pbc", tag="pbc")
                nc.scalar.copy(pbc[:, :], pb[:, :])
                nc.vector.tensor_sub(
                    yfr[:, :, 4 + 2 * j], pa[:, 0:256], pbc[:, 256:512]
                )
                nc.vector.tensor_add(
                    yfr[:, :, 5 + 2 * j], pa[:, 256:512], pbc[:, 0:256]
                )
            for js in range(4):
                pa = pspool.tile([128, 512], F32, name="pas", tag="pa")
                for h in range(2):
                    rhs = ws[:, 256 * (2 * js + h): 256 * (2 * js + h) + 256]
                    nc.tensor.matmul(
                        pa[:, 0:256], xf3r[:, h, js, :], rhs,
                        start=(h == 0), stop=(h == 1 and js != 0),
                    )
                if js == 0:
                    nc.tensor.matmul(
                        pa[:, 0:256], on[:, :], br[:, :],
                        start=False, stop=True,
                    )
                nc.vector.tensor_copy(yfr[:, :, js], pa[:, 0:256])

            # ---- stage E: xbar transpose + inverse cell-DFT ----
            pe = None
            for qp in range(128):
                yft = ytpool.tile([128, 128], BF, name="yft", tag="yft")
                nc.sync.dma_start_transpose(
                    yft[:, :], yf[:, 128 * qp: 128 * qp + 128]
                )
                if qp % 4 == 0:
                    pe = pspool.tile([128, 512], F32, name="pe", tag="psE")
                nc.tensor.matmul(
                    pe[:, 128 * (qp % 4): 128 * (qp % 4) + 128],
                    yft[:, :], cik[:, :],
                    start=True, stop=True,
                )
                if qp % 4 == 3:
                    quad = qp // 4
                    dst = yo[:, 512 * quad: 512 * quad + 512]
                    if quad % 2:
                        nc.scalar.copy(dst, pe[:, :])
                    else:
                        nc.vector.tensor_copy(dst, pe[:, :])
                if qp % 16 == 15:
                    blk = qp // 16
                    nc.scalar.dma_start(
                        out=y_d[:, 2048 * blk: 2048 * blk + 2048],
                        in_=yo[:, 2048 * blk: 2048 * blk + 2048],
                    )
    nc.compile()
    return nc


def kernel(**inputs):
    x = np.asarray(inputs["x"], np.float32)
    kern = np.asarray(inputs["kernel"], np.float32)
    bias = np.asarray(inputs["bias"], np.float32)
    mapping = np.asarray(inputs["mapping"])
    from concourse.bass_utils import run_bass_kernel_spmd

    if "nc" not in _CACHE:
        _CACHE["nc"] = _build_program()
    nc = _CACHE["nc"]
    consts = host_constants(kern, bias, mapping)
    in_maps = []
    for c in range(N_CORES):
        m = dict(consts)
        m["x"] = host_prep_x(x[c * BC: (c + 1) * BC])
        in_maps.append(m)
    res = run_bass_kernel_spmd(nc, in_maps, list(range(N_CORES)))
    _CACHE["last_exec_ns"] = res.exec_time_ns
    y = np.concatenate(
        [host_unpack_y(res.results[c]["y"]) for c in range(N_CORES)], 0
    )
    return np.ascontiguousarray(y.astype(np.float32))


# revision 16
# speedup vs baseline: 3.6416x; 2.8269x over previous
"""DenseEquivariantFFT Trainium2 kernel (batch-sharded over 8 cores), v2.

Math: y = IDFT2_cells( sum_{i,s1} DFT2_cells(x)[b,i,s1,f] * KF[o,i,s1,s2,f] ) + bias
where KF = DFT2_cells(kernel[..,mapping]) and f runs over the 64 cell
frequencies in a real (cos/sin) basis.

Device dataflow per core (128 batches), all bf16 with f32 PSUM accum:
 - host pre-transposes x into [(s1-parity, cell), (t, b4, sp, i)] layout,
   so no on-device input transposes are needed.
 - stage C: per batch, one matmul [K=128=(par,c)] x blockdiag(Cf) -> XF
   with partitions (sp,i) and free (batch, parity, fc).
 - stage D: per frequency pair, 4 matmuls [K=128, N=512] against
   deduplicated [kr|ki] weights; re/im recombined on the vector engine
   into an fc-major yf (contiguous writes).
 - stage E: PE transposes (8 per PSUM bank) put (q2,fc) on partitions,
   then one matmul per q-pair against blockdiag(Ci) produces spatial
   output with batch back on partitions; host un-permutes the layout.
"""
import numpy as np
import ml_dtypes

N_CORES = 8
B, CIN, COUT, NS, NCELL, G = 1024, 32, 32, 8, 64, 512
BC = B // N_CORES  # 128 batches per core

_CACHE = {}


def _freq_classes():
    singles, reps = [], []
    for ku in range(8):
        for kv in range(8):
            f = ku * 8 + kv
            cf = ((-ku) % 8) * 8 + ((-kv) % 8)
            if cf == f:
                singles.append(f)
            elif f < cf:
                reps.append(f)
    return singles, reps  # 4, 30


def _transforms():
    singles, reps = _freq_classes()
    u, v = np.meshgrid(np.arange(8), np.arange(8), indexing="ij")

    def theta(f):
        ku, kv = divmod(f, 8)
        return 2 * np.pi * (ku * u + kv * v) / 8

    Cf = np.zeros((64, 64))
    Ci = np.zeros((64, 64))
    for j, f in enumerate(singles):
        Cf[:, j] = np.cos(theta(f)).ravel()
        Ci[j, :] = np.cos(theta(f)).ravel() / 64
    for j, f in enumerate(reps):
        Cf[:, 4 + 2 * j] = np.cos(theta(f)).ravel()
        Cf[:, 5 + 2 * j] = -np.sin(theta(f)).ravel()
        Ci[4 + 2 * j, :] = 2 * np.cos(theta(f)).ravel() / 64
        Ci[5 + 2 * j, :] = -2 * np.sin(theta(f)).ravel() / 64
    return Cf, Ci, singles, reps


def host_constants(kern, bias, mapping):
    """Device weight tensors. W rows use r=(sp,i) with s1=2*sp+h (parity
    halves); W cols use q=(s2,o)."""
    Cf, Ci, singles, reps = _transforms()
    Kexp = kern[:, :, mapping.reshape(NS, NS, NCELL)]  # [o,i,s1,s2,c]
    KF = np.fft.fft2(
        Kexp.reshape(COUT, CIN, NS, NS, 8, 8).astype(np.float64), axes=(-2, -1)
    ).reshape(COUT, CIN, NS, NS, NCELL)

    wp = np.zeros((64, 128, 512), np.float64)  # unit = 2*j + h
    for j, f in enumerate(reps):
        A = KF[..., f]  # [o,i,s1,s2]
        krf = A.real.transpose(2, 1, 3, 0).reshape(NS, CIN, NS * COUT)
        kif = A.imag.transpose(2, 1, 3, 0).reshape(NS, CIN, NS * COUT)
        for h in range(2):
            kr = krf[h::2].reshape(128, 256)
            ki = kif[h::2].reshape(128, 256)
            wp[2 * j + h] = np.concatenate([kr, ki], axis=1)
    ws = np.zeros((8, 128, 256), np.float64)  # unit = 2*js + h
    for js, f in enumerate(singles):
        A = KF[..., f].real.transpose(2, 1, 3, 0).reshape(NS, CIN, NS * COUT)
        for h in range(2):
            ws[2 * js + h] = A[h::2].reshape(128, 256)

    bias_row = 64.0 * np.tile(bias.ravel().astype(np.float64), NS)[None, :]
    bf = ml_dtypes.bfloat16
    return {
        "CfK": np.kron(np.eye(2), Cf).astype(bf),          # [128,128]
        "CiK": np.kron(np.eye(2), Ci).astype(bf),          # [128,128]
        "Wp": np.ascontiguousarray(
            wp.reshape(4, 16, 128, 512).transpose(0, 2, 1, 3)
        ).reshape(4, 128, 16 * 512).astype(bf),
        "Ws": np.ascontiguousarray(
            ws.transpose(1, 0, 2)
        ).reshape(128, 8 * 256).astype(bf),
        "bias_row": bias_row.astype(bf),
        "ones1": np.ones((1, 128), bf),
        "ident": np.eye(128).astype(bf),
    }


def host_prep_x(xc):
    """[128,32,512] f32 -> [128=(par,c), 16384=(t,b4,sp,i)] bf16."""
    xs = xc.reshape(32, 4, CIN, NCELL, 4, 2)  # t,b4,i,c,sp,par
    xt2 = xs.transpose(5, 3, 0, 1, 4, 2).reshape(128, 16384)
    return np.ascontiguousarray(xt2.astype(ml_dtypes.bfloat16))


def host_unpack_y(yo):
    """[128, 16384=(qp,q2,c)] bf16 -> [128, 32, 512] f32; q=(s2,o)=q2*128+qp."""
    arr = np.asarray(yo, np.float32).reshape(BC, 128, 2, 64)   # b, qp, q2, c
    arr = arr.transpose(0, 2, 1, 3).reshape(BC, 256, 64)       # b, q, c
    arr = arr.reshape(BC, NS, COUT, NCELL).transpose(0, 2, 3, 1)  # b,o,c,s2
    return np.ascontiguousarray(arr).reshape(BC, COUT, G)


def host_simulate(x, kern, bias, mapping):
    """f64 numpy mirror of the device algebra (layout validation)."""
    Cf, Ci, singles, reps = _transforms()
    Kexp = kern[:, :, mapping.reshape(NS, NS, NCELL)]
    KF = np.fft.fft2(
        Kexp.reshape(COUT, CIN, NS, NS, 8, 8).astype(np.float64), axes=(-2, -1)
    ).reshape(COUT, CIN, NS, NS, NCELL)
    xs = x.reshape(B, CIN, NCELL, NS).astype(np.float64)
    XF = np.einsum("bics,cf->bisf", xs, Cf)  # [b,i,s1,fc]
    yf = np.zeros((B, NS, COUT, 64))  # [b,s2,o,fc]
    for j, f in enumerate(reps):
        A = KF[..., f]
        yf[..., 4 + 2 * j] = (
            np.einsum("bis,oist->bto", XF[..., 4 + 2 * j], A.real)
            - np.einsum("bis,oist->bto", XF[..., 5 + 2 * j], A.imag)
        )
        yf[..., 5 + 2 * j] = (
            np.einsum("bis,oist->bto", XF[..., 4 + 2 * j], A.imag)
            + np.einsum("bis,oist->bto", XF[..., 5 + 2 * j], A.real)
        )
    for js, f in enumerate(singles):
        yf[..., js] = np.einsum("bis,oist->bto", XF[..., js], KF[..., f].real)
    yf[..., 0] += 64.0 * bias.ravel()[None, None, :]
    y = np.einsum("btof,fc->btoc", yf, Ci)  # [b,s2,o,c]
    y = y.transpose(0, 2, 3, 1).reshape(B, COUT, G)
    return y.astype(np.float32)


def _build_program():
    import concourse.bass as bass
    import concourse.bacc as bacc
    import concourse.mybir as mybir
    from concourse.tile import TileContext

    BF = mybir.dt.bfloat16
    F32 = mybir.dt.float32
    nc = bacc.Bacc("TRN2", target_bir_lowering=False, debug=False,
                   num_devices=N_CORES)
    x_d = nc.dram_tensor("x", [128, 16384], BF, kind="ExternalInput")
    cfk_d = nc.dram_tensor("CfK", [128, 128], BF, kind="ExternalInput")
    cik_d = nc.dram_tensor("CiK", [128, 128], BF, kind="ExternalInput")
    wp_d = nc.dram_tensor("Wp", [4, 128, 8192], BF, kind="ExternalInput")
    ws_d = nc.dram_tensor("Ws", [128, 2048], BF, kind="ExternalInput")
    br_d = nc.dram_tensor("bias_row", [1, 256], BF, kind="ExternalInput")
    on_d = nc.dram_tensor("ones1", [1, 128], BF, kind="ExternalInput")
    id_d = nc.dram_tensor("ident", [128, 128], BF, kind="ExternalInput")
    y_d = nc.dram_tensor("y", [128, 16384], BF, kind="ExternalOutput")

    with TileContext(nc) as tc:
        with (
            tc.tile_pool(name="const", bufs=1) as cpool,
            tc.tile_pool(name="xt", bufs=1) as xtpool,
            tc.tile_pool(name="xf3", bufs=1) as xfpool,
            tc.tile_pool(name="w", bufs=1) as wpool,
            tc.tile_pool(name="yf", bufs=1) as yfpool,
            tc.tile_pool(name="yo", bufs=1) as yopool,
            tc.tile_pool(name="pbc", bufs=3) as pbcpool,
            tc.tile_pool(name="yt", bufs=2) as ytpool,
            tc.tile_pool(name="ps", bufs=2, space="PSUM") as pspool,
        ):
            cfk = cpool.tile([128, 128], BF, name="cfk")
            nc.sync.dma_start(out=cfk[:, :], in_=cfk_d[:, :])
            cik = cpool.tile([128, 128], BF, name="cik")
            nc.sync.dma_start(out=cik[:, :], in_=cik_d[:, :])
            br = cpool.tile([1, 256], BF, name="br")
            nc.sync.dma_start(out=br[:, :], in_=br_d[:, :])
            on = cpool.tile([1, 128], BF, name="on")
            nc.sync.dma_start(out=on[:, :], in_=on_d[:, :])
            ident = cpool.tile([128, 128], BF, name="ident")
            nc.sync.dma_start(out=ident[:, :], in_=id_d[:, :])

            xt = [xtpool.tile([128, 4096], BF, name=f"xt{g}", tag=f"xt{g}")
                  for g in range(4)]
            for g in range(4):
                nc.sync.dma_start(
                    out=xt[g][:, :], in_=x_d[:, 4096 * g: 4096 * (g + 1)]
                )
            # weights on the ACT HWDGE ring, parallel with x loads
            wsb = [wpool.tile([128, 8192], BF, name=f"wp{k}", tag=f"wp{k}")
                   for k in range(4)]
            for k in range(4):
                nc.scalar.dma_start(out=wsb[k][:, :], in_=wp_d[k])
            ws = wpool.tile([128, 2048], BF, name="ws", tag="ws")
            nc.scalar.dma_start(out=ws[:, :], in_=ws_d[:, :])

            xf3 = xfpool.tile([128, 16384], BF, name="xf3")
            yf = yfpool.tile([128, 16384], BF, name="yf")
            yo = yopool.tile([128, 16384], BF, name="yo")

            # ---- stage C: forward cell-DFT, one matmul per batch ----
            for t in range(32):
                g, tl = divmod(t, 8)
                xtr = xt[g][:, :].rearrange(
                    "p (t b4 r) -> p t b4 r", t=8, b4=4
                )
                pc = pspool.tile([128, 512], F32, name="pc", tag="psA")
                for b4 in range(4):
                    nc.tensor.matmul(
                        pc[:, 128 * b4: 128 * b4 + 128],
                        xtr[:, tl, b4, :],
                        cfk[:, :],
                        start=True, stop=True,
                    )
                dst = xf3[:, :].rearrange("p (t q) -> p t q", t=32)
                if t % 2:
                    nc.scalar.copy(dst[:, t, :], pc[:, :])
                else:
                    nc.vector.tensor_copy(dst[:, t, :], pc[:, :])

            # ---- stage D: per-frequency mixing ----
            # yf free = (m, qp) with m = q2*64+fc, q = q2*128+qp: combine
            # writes land as two contiguous 128-runs, and stage-E chunk qp
            # is a single stride-128 run (legal stationary-operand AP).
            xf3r = xf3[:, :].rearrange(
                "p (b s2 fc) -> p s2 fc b", s2=2, fc=64
            )
            yfm = yf[:, :].rearrange(
                "p (q2 fc qp) -> p q2 fc qp", q2=2, fc=64
            )
            for j in range(30):
                k, unit0 = divmod(2 * j, 16)
                pa = pspool.tile([128, 512], F32, name="pa", tag="pa")
                pb = pspool.tile([128, 512], F32, name="pb", tag="pb")
                for h in range(2):
                    rhs = wsb[k][:, 512 * (unit0 + h): 512 * (unit0 + h) + 512]
                    nc.tensor.matmul(
                        pa[:, :], xf3r[:, h, 4 + 2 * j, :], rhs,
                        start=(h == 0), stop=(h == 1),
                    )
                for h in range(2):
                    rhs = wsb[k][:, 512 * (unit0 + h): 512 * (unit0 + h) + 512]
                    nc.tensor.matmul(
                        pb[:, :], xf3r[:, h, 5 + 2 * j, :], rhs,
                        start=(h == 0), stop=(h == 1),
                    )
                pbc = pbcpool.tile([128, 512], BF, name="pbc", tag="pbc")
                nc.scalar.copy(pbc[:, :], pb[:, :])
                fr, fi = 4 + 2 * j, 5 + 2 * j
                par = pa[:, :].rearrange("p (ri q2 qp) -> p ri q2 qp", ri=2, q2=2)
                pbr = pbc[:, :].rearrange("p (ri q2 qp) -> p ri q2 qp", ri=2, q2=2)
                nc.vector.tensor_sub(
                    yfm[:, :, fr, :], par[:, 0], pbr[:, 1]
                )
                nc.vector.tensor_add(
                    yfm[:, :, fi, :], par[:, 1], pbr[:, 0]
                )
            for js in range(4):
                pa = pspool.tile([128, 512], F32, name="pas", tag="pa")
                for h in range(2):
                    rhs = ws[:, 256 * (2 * js + h): 256 * (2 * js + h) + 256]
                    nc.tensor.matmul(
                        pa[:, 0:256], xf3r[:, h, js, :], rhs,
                        start=(h == 0), stop=(h == 1 and js != 0),
                    )
                if js == 0:
                    nc.tensor.matmul(
                        pa[:, 0:256], on[:, :], br[:, :],
                        start=False, stop=True,
                    )
                pasr = pa[:, 0:256].rearrange("p (q2 qp) -> p q2 qp", q2=2)
                nc.vector.tensor_copy(yfm[:, :, js, :], pasr[:, :, :])

            # ---- stage E: PE transpose + inverse cell-DFT ----
            # chunk qp = cols {qp + 128*m'}: single stride-128 run
            yfq = yf[:, :].rearrange("p (m qp) -> p qp m", m=128)
            pe = None
            for grp in range(16):
                pt = pspool.tile([128, 1024], BF, name="pt", tag="psT")
                for k in range(8):
                    qp = 8 * grp + k
                    nc.tensor.transpose(
                        pt[:, 128 * k: 128 * k + 128],
                        yfq[:, qp, :],
                        ident[:, :],
                    )
                yt = ytpool.tile([128, 1024], BF, name="yt", tag="yt")
                if grp % 2:
                    nc.scalar.copy(yt[:, :], pt[:, :])
                else:
                    nc.vector.tensor_copy(yt[:, :], pt[:, :])
                for k in range(8):
                    qp = 8 * grp + k
                    if qp % 4 == 0:
                        pe = pspool.tile(
                            [128, 512], F32, name="pe", tag="psA"
                        )
                    nc.tensor.matmul(
                        pe[:, 128 * (qp % 4): 128 * (qp % 4) + 128],
                        yt[:, 128 * k: 128 * k + 128], cik[:, :],
                        start=True, stop=True,
                    )
                    if qp % 4 == 3:
                        quad = qp // 4
                        dst = yo[:, 512 * quad: 512 * quad + 512]
                        if quad % 2:
                            nc.scalar.copy(dst, pe[:, :])
                        else:
                            nc.vector.tensor_copy(dst, pe[:, :])
                if grp % 2 == 1:
                    blk = grp // 2
                    nc.scalar.dma_start(
                        out=y_d[:, 2048 * blk: 2048 * blk + 2048],
                        in_=yo[:, 2048 * blk: 2048 * blk + 2048],
                    )
    nc.compile()
    return nc


def kernel(**inputs):
    x = np.asarray(inputs["x"], np.float32)
    kern = np.asarray(inputs["kernel"], np.float32)
    bias = np.asarray(inputs["bias"], np.float32)
    mapping = np.asarray(inputs["mapping"])
    from concourse.bass_utils import run_bass_kernel_spmd

    if "nc" not in _CACHE:
        _CACHE["nc"] = _build_program()
    nc = _CACHE["nc"]
    consts = host_constants(kern, bias, mapping)
    in_maps = []
    for c in range(N_CORES):
        m = dict(consts)
        m["x"] = host_prep_x(x[c * BC: (c + 1) * BC])
        in_maps.append(m)
    res = run_bass_kernel_spmd(nc, in_maps, list(range(N_CORES)))
    _CACHE["last_exec_ns"] = res.exec_time_ns
    y = np.concatenate(
        [host_unpack_y(res.results[c]["y"]) for c in range(N_CORES)], 0
    )
    return np.ascontiguousarray(y.astype(np.float32))
